# revision 6
# baseline (speedup 1.0000x reference)
"""Masked attention kernel for Trainium2, 8 NeuronCores.

Problem: B=2, H=8, S=4096, D=64 attention with a shared boolean mask
(True = masked out with -1e9 BEFORE the /sqrt(D) scaling), softmax over
keys, then @ V.

Sharding: batch*head parallel. B*H = 16 fused heads, 2 per core. The
mask is shared by every head, so each core's inner loop interleaves its
two heads over one streamed copy of the mask.

Device algorithm (per core, everything transposed so that keys sit on
PSUM/SBUF partitions):
  S^T[k,q] = K @ Q^T        (PE, fp16 inputs, fp32 PSUM)
  P^T      = exp(S^T / 8)   (ACT, PSUM -> SBUF fp16)
  P^T     *= keepT          (DVE fp16 2x mode; keepT = ~mask^T as fp16 0/1)
  [O^T;sum] = [V|1]^T-style accumulation: lhsT=[V,ones], rhs=P^T (PE)
  O = transpose(O^T) / sum  (PE transpose + DVE reciprocal/scale)

exp(-1e9/8)*anything == 0 == exp(x)*0, so multiplying the exp by the
keep mask is exactly equivalent to the reference's -1e9 fill (softmax is
shift invariant; no row is fully masked for this data distribution).
"""

import os
import sys

import numpy as np

for _p in ("/opt/trn_rl_repo",):
    if os.path.isdir(_p) and _p not in sys.path:
        sys.path.insert(0, _p)

B, H, S, D = 2, 8, 4096, 64
N_CORES = 8
HPC = (B * H) // N_CORES  # heads per core = 2
QSTRIP = 512  # query columns processed per outer strip
KSUP = 2  # key-tiles (128 keys each) per exp/mask chunk


def build_program(s=S, hpc=HPC, qstrip=QSTRIP, ksup=KSUP, reps=1):
    import concourse.bacc as bacc
    import concourse.mybir as mybir
    import concourse.tile as tile
    from concourse.masks import make_identity

    f16 = mybir.dt.float16
    f32 = mybir.dt.float32
    Exp = mybir.ActivationFunctionType.Exp

    ktiles = s // 128  # number of 128-key tiles
    nsub = qstrip // 128  # 128-query subtiles per strip
    nsup = ktiles // ksup  # key supertiles per strip
    nstrip = s // qstrip

    nc = bacc.Bacc(None, target_bir_lowering=False)
    QT = nc.dram_tensor("QT", [64, hpc, s], f16, kind="ExternalInput")
    KTD = nc.dram_tensor("KTD", [64, hpc, s], f16, kind="ExternalInput")
    VP = nc.dram_tensor("VP", [128, hpc, ktiles, 65], f16, kind="ExternalInput")
    KEEP = nc.dram_tensor("KEEP", [s, s], f16, kind="ExternalInput")
    O = nc.dram_tensor("O", [hpc, s, 64], f32, kind="ExternalOutput")

    with tile.TileContext(nc) as tc:
        with (
            tc.tile_pool(name="persist", bufs=1) as persist,
            tc.tile_pool(name="keepp", bufs=2) as keepp,
            tc.tile_pool(name="pp", bufs=3) as pp,
            tc.tile_pool(name="sp", bufs=3, space="PSUM") as sp,
            tc.tile_pool(name="op", bufs=2, space="PSUM") as op,
            tc.tile_pool(name="ep", bufs=2) as ep,
        ):
            qt = persist.tile([64, hpc, s], f16)
            nc.sync.dma_start(out=qt, in_=QT[:])
            kt = persist.tile([64, hpc, s], f16)
            nc.sync.dma_start(out=kt, in_=KTD[:])
            vt = persist.tile([128, hpc, ktiles, 65], f16)
            nc.sync.dma_start(out=vt, in_=VP[:])
            ident = persist.tile([128, 128], f32)
            make_identity(nc, ident)

            for qs in range(nstrip * reps):
                qs = qs % nstrip
                q0 = qs * qstrip
                # one strip of the keep mask, shared by both heads:
                # [key%128, keytile, q]
                kstrip = keepp.tile([128, ktiles, qstrip], f16)
                nc.sync.dma_start(
                    out=kstrip,
                    in_=KEEP[:, q0 : q0 + qstrip].rearrange(
                        "(j p) q -> p j q", p=128
                    ),
                )
                oaccs = [
                    op.tile([65, qstrip], f32, tag="oa", name=f"oacc{h}")
                    for h in range(hpc)
                ]
                for ks in range(nsup):
                    for h in range(hpc):
                        st = sp.tile([128, ksup, qstrip], f32)
                        for j in range(ksup):
                            k0 = (ks * ksup + j) * 128
                            nc.tensor.matmul(
                                st[:, j, :],
                                lhsT=kt[:, h, k0 : k0 + 128],
                                rhs=qt[:, h, q0 : q0 + qstrip],
                                start=True,
                                stop=True,
                            )
                        pt = pp.tile([128, ksup, qstrip], f16)
                        nc.scalar.activation(pt, st, Exp, scale=0.125)
                        nc.vector.tensor_mul(
                            pt, pt, kstrip[:, ks * ksup : (ks + 1) * ksup, :]
                        )
                        for j in range(ksup):
                            ktile = ks * ksup + j
                            nc.tensor.matmul(
                                oaccs[h],
                                lhsT=vt[:, h, ktile, :],
                                rhs=pt[:, j, :],
                                start=(ktile == 0),
                                stop=(ktile == ktiles - 1),
                            )
                for h in range(hpc):
                    osb = ep.tile([65, qstrip], f32)
                    nc.vector.tensor_copy(osb, oaccs[h])
                    ott = op.tile([128, nsub, 65], f32, tag="oa")
                    for t in range(nsub):
                        nc.tensor.transpose(
                            ott[:, t, :],
                            osb[:, t * 128 : (t + 1) * 128],
                            ident[:65, :65],
                        )
                    rec = ep.tile([128, nsub], f32)
                    nc.vector.reciprocal(rec, ott[:, :, 64])
                    of = ep.tile([128, nsub, 64], f32)
                    for t in range(nsub):
                        nc.vector.tensor_scalar_mul(
                            of[:, t, :], ott[:, t, :64], rec[:, t : t + 1]
                        )
                    nc.sync.dma_start(
                        out=O[h, q0 : q0 + qstrip, :].rearrange(
                            "(t p) d -> p t d", p=128
                        ),
                        in_=of,
                    )
    nc.compile()
    return nc


def prep_inputs(Q, K, V, mask, s=S, hpc=HPC):
    """Host-side marshalling: fp16 casts, transposes, per-core shards."""
    nheads = Q.shape[0] * Q.shape[1]
    ncores = nheads // hpc
    Qr = np.asarray(Q, dtype=np.float32).reshape(nheads, s, D)
    Kr = np.asarray(K, dtype=np.float32).reshape(nheads, s, D)
    Vr = np.asarray(V, dtype=np.float32).reshape(nheads, s, D)
    keepT = np.ascontiguousarray(
        (~np.asarray(mask).reshape(s, s)).T.astype(np.float16)
    )
    in_maps = []
    for c in range(ncores):
        sl = slice(c * hpc, (c + 1) * hpc)
        # [hpc, S, D] -> [D(=64 partitions), hpc, S]
        qtc = np.ascontiguousarray(
            Qr[sl].transpose(2, 0, 1).astype(np.float16)
        )
        ktc = np.ascontiguousarray(
            Kr[sl].transpose(2, 0, 1).astype(np.float16)
        )
        # [hpc, S, D] -> [128, hpc, S//128, 65] with ones in column 64
        vpc = np.ones((128, hpc, s // 128, 65), dtype=np.float16)
        vpc[:, :, :, :64] = (
            Vr[sl].reshape(hpc, s // 128, 128, D).transpose(2, 0, 1, 3)
        ).astype(np.float16)
        in_maps.append({"QT": qtc, "KTD": ktc, "VP": vpc, "KEEP": keepT})
    return in_maps


def gather_outputs(results, s=S, hpc=HPC):
    outs = [r["O"] for r in results]
    full = np.concatenate(outs, axis=0)  # [B*H, S, D]
    return np.ascontiguousarray(full.reshape(B, H, s, D).astype(np.float32))


_CACHE = {}


def get_program():
    if "nc" not in _CACHE:
        _CACHE["nc"] = build_program()
    return _CACHE["nc"]


def kernel(Q, K, V, mask):
    from concourse.bass_utils import run_bass_kernel_spmd

    nc = get_program()
    in_maps = prep_inputs(Q, K, V, mask)
    res = run_bass_kernel_spmd(nc, in_maps, core_ids=list(range(N_CORES)))
    return gather_outputs(res.results)


if __name__ == "__main__":
    import jax

    sys.path.insert(0, os.path.dirname(os.path.abspath(__file__)))
    import reference

    with jax.default_device(jax.devices("cpu")[0]):
        inputs = {k: np.asarray(v) for k, v in reference.setup_inputs().items()}
        expected = np.asarray(reference.reference(**inputs))
    actual = kernel(**inputs)
    err = np.abs(actual - expected).max() / np.abs(expected).max()
    print("Relative error:", err)
